# revision 1
# baseline (speedup 1.0000x reference)
"""Causal self-attention with RoPE on 8 Trainium2 NeuronCores.

Problem (hardcoded): x (4, 2048, 2048) f32, w_attn (2048, 6144),
w_proj (2048, 2048), rope_cos/rope_sin (2048, 64), 16 heads, hd=128.

Sharding: 8 cores = 4 batches x 2 head-groups (8 heads each).  Each core
computes qkv projection for its heads, RoPE, causal attention, and a
partial output projection (its head-group's rows of w_proj).  The host
sums the two partials per batch (the "all-reduce after c_proj") and
transposes back, since the device kernel works fully transposed.

Device layout choices:
  - qT, kT stored [hd=128 partitions, T free]; S^T tiles [j_keys, q]
    come straight from matmul(lhsT=kT_j, rhs=qT_q).  Softmax exp is
    elementwise (no max subtraction needed: scores ~ N(0,1), max ~ 6);
    causality = skipping j>q blocks + masking diagonal blocks.  The PV
    matmul consumes P^T directly with v in natural [T, hd] layout as
    lhsT, producing o^T with no transposes anywhere.
  - RoPE pairs (2i, 2i+1) are host-permuted to (i, 64+i) by permuting
    w_attn's q/k columns (dot products are permutation invariant), so
    the rotation acts on contiguous partition ranges.
  - All matmuls run in float32r (fp32 bits, full-rate PE mode,
    ~1.5e-4 scale-relative error measured on K=2048).
"""

import sys

sys.path.insert(0, "/opt/trn_rl_repo")

import numpy as np

import concourse.bass as bass
import concourse.mybir as mybir
import concourse.tile as tile

F32 = mybir.dt.float32
F32R = mybir.dt.float32r
P = 128


# --------------------------------------------------------------------------
# This container's walrus build rejects any instruction carrying more than
# one sem wait.  Split extras onto NoOps inserted before the instruction on
# the same engine (per-engine program order makes the waits complete first).
def _split_multi_waits(nc):
    n = 0
    for fn in nc.m.functions:
        for bb in fn.blocks:
            out = []
            changed = False
            for inst in bb.instructions:
                si = inst.sync_info
                waits = list(si.on_wait or []) if si is not None else []
                if len(waits) > 1:
                    changed = True
                    n += 1
                    for w in waits[:-1]:
                        nop = mybir.InstNoOp(
                            name=nc.get_next_instruction_name(),
                            engine=inst.engine,
                            ins=[],
                            outs=[],
                            sync_info=mybir.SyncInfo(on_wait=[w], on_update=[]),
                        )
                        try:
                            nc.register_instruction(nop, overwrite=True)
                        except Exception:
                            pass
                        out.append(nop)
                    inst.sync_info = mybir.SyncInfo(
                        on_wait=[waits[-1]], on_update=list(si.on_update or [])
                    )
                out.append(inst)
            if changed:
                bb.instructions = out
    return n


def _r(ap):
    return ap.bitcast(F32R)


def build_attention_core(T=2048, C=2048, G=8, n_half=2):
    """One core\'s program.  T tokens, C model dim, G heads in this core\'s
    group (hd=128 each).  Returns the Bass object."""
    KO = C // P          # contraction tiles over model dim
    TH = T // n_half     # tokens per phase-A pass
    NTC = max(TH // 512, 1)   # 512-wide t chunks per half (phase A qk)
    TCW = min(TH, 512)
    NTB = TH // P        # 128-tall t blocks per half (phase A v)
    VN = min(512, G * P)  # v column chunk
    NV = (G * P) // VN
    NQ = max(T // 512, 1)  # 512-wide q chunks (phase B)
    QW = min(T, 512)
    JPQ = QW // P        # j tiles per q chunk width
    NJ = T // P          # total j tiles
    KQ = max(KO // 4, 1)  # kc per xt quarter tile
    NXQ = KO // KQ

    nc = bass.Bass()
    xt = nc.dram_tensor("xt", [n_half, P, KO, TH], F32R, kind="ExternalInput")
    wqk = nc.dram_tensor("wqk", [2 * G, P, KO, P], F32R, kind="ExternalInput")
    wv = nc.dram_tensor("wv", [NV, P, KO, VN], F32R, kind="ExternalInput")
    wp = nc.dram_tensor("wp", [KO, P, G, P], F32R, kind="ExternalInput")
    # cosd = [cos; cos], sind = [-sin; +sin]  (rope = raw*cosd + swapped*sind)
    cosp = nc.dram_tensor("cosp", [P, T], F32, kind="ExternalInput")
    sinp = nc.dram_tensor("sinp", [P, T], F32, kind="ExternalInput")
    maskt = nc.dram_tensor("maskt", [P, P], F32, kind="ExternalInput")
    ones_s = nc.dram_tensor("ones_s", [P, 1], F32R, kind="ExternalInput")
    ones1 = nc.dram_tensor("ones1", [1, P], F32R, kind="ExternalInput")
    outT = nc.dram_tensor("outT", [C, T], F32, kind="ExternalOutput")

    scale = 1.0 / np.sqrt(128.0)

    with tile.TileContext(nc) as tc:
        with (
            tc.tile_pool(name="dram", bufs=1, space="DRAM") as dram,
            tc.tile_pool(name="const", bufs=1) as cpool,
        ):
            qkd = dram.tile([2 * G, P, T], F32R)
            od = dram.tile([G, P, T], F32R)

            cos_s = cpool.tile([P, T], F32)
            nc.sync.dma_start(cos_s[:], cosp[:])
            sin_s = cpool.tile([P, T], F32)
            nc.sync.dma_start(sin_s[:], sinp[:])
            mask_s = cpool.tile([P, P], F32)
            nc.sync.dma_start(mask_s[:], maskt[:])
            one_col = cpool.tile([P, 1], F32R)
            nc.sync.dma_start(one_col[:], ones_s[:])
            one_row = cpool.tile([1, P], F32R)
            nc.sync.dma_start(one_row[:], ones1[:])

            with tc.tile_pool(name="vall", bufs=1) as va_pool:
                # v stays resident in SBUF through phases A and B:
                # v_all[ti, to, hh*128+d] = v[to*128+ti, head hh, d]
                v_all = va_pool.tile([P, NJ, G * P], F32R, tag="vall")

                # ------------- Phase A: qkT + RoPE, v -------------
                with (
                    tc.tile_pool(name="xt", bufs=4) as xt_pool,
                    tc.tile_pool(name="wqk", bufs=2) as wqk_pool,
                    tc.tile_pool(name="wv", bufs=1) as wv_pool,
                    tc.tile_pool(name="qkraw", bufs=1) as qkraw_pool,
                    tc.tile_pool(name="roped", bufs=1) as roped_pool,
                    tc.tile_pool(name="ropetmp", bufs=1) as rtmp_pool,
                    tc.tile_pool(name="psA", bufs=2, space="PSUM") as psA,
                    tc.tile_pool(name="psV", bufs=2, space="PSUM") as psV,
                ):
                    for H in range(n_half):
                        t0 = H * TH
                        xtq = []
                        for qq in range(NXQ):
                            xq = xt_pool.tile([P, KQ, TH], F32R, tag="xtq",
                                              name=f"xtq{qq}")
                            nc.sync.dma_start(
                                xq[:], xt[H, :, qq * KQ : (qq + 1) * KQ, :]
                            )
                            xtq.append(xq)
                        # --- q,k heads ---
                        for m in range(2 * G):
                            w_s = wqk_pool.tile([P, KO, P], F32R, tag="wqk")
                            nc.sync.dma_start(w_s[:], wqk[m])
                            pss = [
                                psA.tile([P, TCW], F32, tag=f"pqk{i}",
                                         name=f"pqk{i}")
                                for i in range(NTC)
                            ]
                            for kc in range(KO):
                                for i in range(NTC):
                                    nc.tensor.matmul(
                                        pss[i][:],
                                        w_s[:, kc, :],
                                        xtq[kc // KQ][
                                            :, kc % KQ,
                                            i * TCW : (i + 1) * TCW,
                                        ],
                                        start=(kc == 0),
                                        stop=(kc == KO - 1),
                                    )
                            # RoPE: rope = raw*[cos;cos] + swap(raw)*[-sin;+sin]
                            raw = qkraw_pool.tile([P, TH], F32, tag="qkraw")
                            sw = rtmp_pool.tile([P, TH], F32, tag="rtmp")
                            rop = roped_pool.tile([P, TH], F32R, tag="roped")
                            for i in range(NTC):
                                sl = slice(i * TCW, (i + 1) * TCW)
                                nc.any.tensor_copy(raw[:, sl], pss[i][:])
                                nc.vector.tensor_mul(
                                    rop[:, sl], pss[i][:],
                                    cos_s[:, t0 + i * TCW : t0 + (i + 1) * TCW],
                                )
                            nc.sync.dma_start(sw[0:64, :], raw[64:128, :])
                            nc.sync.dma_start(sw[64:128, :], raw[0:64, :])
                            nc.vector.tensor_mul(
                                sw[:], sw[:], sin_s[:, t0 : t0 + TH]
                            )
                            nc.vector.tensor_add(rop[:], rop[:], sw[:])
                            nc.sync.dma_start(qkd[m, :, t0 : t0 + TH], rop[:])
                        # --- v (kept in SBUF, natural [t, d] layout) ---
                        for n2 in range(NV):
                            wv_s = wv_pool.tile([P, KO, VN], F32R, tag="wv")
                            nc.sync.dma_start(wv_s[:], wv[n2])
                            for tb in range(NTB):
                                psv = psV.tile([P, VN], F32, tag="pv")
                                for kc in range(KO):
                                    nc.tensor.matmul(
                                        psv[:],
                                        xtq[kc // KQ][
                                            :, kc % KQ, tb * P : (tb + 1) * P
                                        ],
                                        wv_s[:, kc, :],
                                        start=(kc == 0),
                                        stop=(kc == KO - 1),
                                    )
                                nc.any.tensor_copy(
                                    v_all[
                                        :, H * NTB + tb,
                                        n2 * VN : (n2 + 1) * VN,
                                    ],
                                    psv[:],
                                )

                # ------------- Phase B: attention per head -------------
                with (
                    tc.tile_pool(name="qh", bufs=3) as q_pool,
                    tc.tile_pool(name="kh", bufs=3) as k_pool,
                    tc.tile_pool(name="pt", bufs=6) as pt_pool,
                    tc.tile_pool(name="racc", bufs=2) as racc_pool,
                    tc.tile_pool(name="rsb", bufs=2) as rsb_pool,
                    tc.tile_pool(name="rinv", bufs=2) as rinv_pool,
                    tc.tile_pool(name="rq", bufs=2) as rq_pool,
                    tc.tile_pool(name="dramq", bufs=2, space="DRAM") as dramq,
                    tc.tile_pool(name="oacc", bufs=2) as oacc_pool,
                    tc.tile_pool(name="psS", bufs=5, space="PSUM") as psS,
                    tc.tile_pool(name="psO", bufs=1, space="PSUM") as psO,
                    tc.tile_pool(name="psR", bufs=1, space="PSUM") as psR,
                    tc.tile_pool(name="psRep", bufs=1, space="PSUM") as psRep,
                ):
                    for h in range(G):
                        qT = q_pool.tile([P, T], F32R, tag="q")
                        nc.sync.dma_start(qT[:], qkd[h])
                        kT = k_pool.tile([P, T], F32R, tag="k")
                        nc.sync.dma_start(kT[:], qkd[G + h])
                        oT = oacc_pool.tile([P, T], F32R, tag="oacc")
                        for Q in range(NQ):
                            jmax = JPQ * (Q + 1) - 1  # inclusive
                            racc = racc_pool.tile([P, QW], F32R, tag="racc")
                            pso = psO.tile([P, QW], F32, tag="pso")
                            # PV(J) consumes exp(S(J)) from ACT; emit it LOOK
                            # S-matmuls later so PE never stalls on ACT.
                            LOOK = 3
                            pend = []
                            for J in range(jmax + 1 + LOOK):
                                if J <= jmax:
                                    k_d = J - JPQ * Q  # diag idx if >= 0
                                    co = max(k_d, 0) * P
                                    pss = psS.tile([P, QW], F32, tag="pss")
                                    nc.tensor.matmul(
                                        pss[:, co:],
                                        kT[:, J * P : (J + 1) * P],
                                        qT[:, Q * QW + co : (Q + 1) * QW],
                                        start=True,
                                        stop=True,
                                        skip_group_check=True,
                                    )
                                    pT = pt_pool.tile([P, QW], F32R, tag="pt")
                                    nc.scalar.activation(
                                        pT[:, co:], pss[:, co:],
                                        mybir.ActivationFunctionType.Exp,
                                        scale=scale,
                                    )
                                    if k_d >= 0:
                                        nc.vector.tensor_mul(
                                            pT[:, co : co + P],
                                            pT[:, co : co + P],
                                            mask_s[:],
                                        )
                                    if J == 0:
                                        nc.any.tensor_copy(racc[:], pT[:])
                                    else:
                                        nc.vector.tensor_add(
                                            racc[:, co:], racc[:, co:],
                                            pT[:, co:],
                                        )
                                    pend.append((J, co, pT))
                                if J >= LOOK:
                                    Jp, cop, pTp = pend.pop(0)
                                    nc.tensor.matmul(
                                        pso[:, cop:],
                                        v_all[:, Jp, h * P : (h + 1) * P],
                                        pTp[:, cop:],
                                        start=(Jp == 0),
                                        stop=(Jp == jmax),
                                        skip_group_check=True,
                                    )
                            psr = psR.tile([1, QW], F32, tag="psr")
                            nc.tensor.matmul(
                                psr[:], one_col[:], racc[:],
                                start=True, stop=True, skip_group_check=True,
                            )
                            rsb = rsb_pool.tile([1, QW], F32, tag="rsb")
                            nc.any.tensor_copy(rsb[:], psr[:])
                            # fold [1,QW] -> [128,QW/128] via DRAM so the
                            # (free-size-bound) reciprocal runs on QW/128
                            # elems per lane instead of QW on one lane
                            FD = QW // P
                            rqda = dramq.tile([P, FD], F32, tag="rqda",
                                              name="rqda")
                            nc.gpsimd.dma_start(
                                rqda.rearrange("a b -> (a b)")[None, :],
                                rsb[:],
                            )
                            rq = rq_pool.tile([P, FD], F32, tag="rq")
                            nc.gpsimd.dma_start(rq[:], rqda[:])
                            rqr = rq_pool.tile([P, FD], F32R, tag="rqr")
                            with nc.allow_low_precision(reason="f32r is 4B"):
                                nc.vector.reciprocal(rqr[:], rq[:])
                            rqdb = dramq.tile([P, FD], F32R, tag="rqdb",
                                              name="rqdb")
                            nc.gpsimd.dma_start(rqdb[:], rqr[:])
                            rinv1 = rsb_pool.tile([1, QW], F32R, tag="rinv1")
                            nc.gpsimd.dma_start(
                                rinv1[:],
                                rqdb.rearrange("a b -> (a b)")[None, :],
                            )
                            psrep = psRep.tile([P, QW], F32, tag="psrep")
                            nc.tensor.matmul(
                                psrep[:], one_row[:], rinv1[:],
                                start=True, stop=True, skip_group_check=True,
                            )
                            rinv = rinv_pool.tile([P, QW], F32, tag="rinv")
                            nc.any.tensor_copy(rinv[:], psrep[:])
                            nc.vector.tensor_mul(
                                oT[:, Q * QW : (Q + 1) * QW], pso[:], rinv[:]
                            )
                        nc.scalar.dma_start(od[h], oT[:])

            # ------------- Phase C: output projection -------------
            with (
                tc.tile_pool(name="wp", bufs=1) as wp_pool,
                tc.tile_pool(name="otc", bufs=2) as otc_pool,
                tc.tile_pool(name="csb", bufs=4) as csb_pool,
                tc.tile_pool(name="psC", bufs=4, space="PSUM") as psC,
            ):
                wp_s = wp_pool.tile([P, KO, G, P], F32R, tag="wp")
                for m in range(KO):
                    nc.sync.dma_start(
                        wp_s[:, m], wp[m]
                    )
                for t in range(NQ):
                    oTt = otc_pool.tile([P, G, QW], F32R, tag="otc")
                    nc.sync.dma_start(
                        oTt[:],
                        od.rearrange("h p t -> p h t")[
                            :, :, t * QW : (t + 1) * QW
                        ],
                    )
                    for m in range(KO):
                        psc = psC.tile([P, QW], F32, tag="psc")
                        for h in range(G):
                            nc.tensor.matmul(
                                psc[:],
                                wp_s[:, m, h, :],
                                oTt[:, h, :],
                                start=(h == 0),
                                stop=(h == G - 1),
                            )
                        csb = csb_pool.tile([P, QW], F32, tag="csb")
                        nc.any.tensor_copy(csb[:], psc[:])
                        nc.sync.dma_start(
                            outT[m * P : (m + 1) * P, t * QW : (t + 1) * QW],
                            csb[:],
                        )

    _split_multi_waits(nc)
    return nc


# --------------------------------------------------------------------------
def _prep_core_inputs(xb, w_attn, w_proj, rope_cos, rope_sin, g, G=8, n_half=2):
    """Host-side shard prep for one core: batch slice xb (T, C), group g."""
    T, C = xb.shape
    KO = C // P
    TH = T // n_half
    VN = min(512, G * P)
    NV = (G * P) // VN
    gc = g * G * P  # column offset of this group within one qkv section

    # x^T arranged [half, ki, ko, t]
    xtT = np.ascontiguousarray(xb.T)  # (C, T)
    xt = np.ascontiguousarray(
        xtT.reshape(KO, P, n_half, TH).transpose(2, 1, 0, 3)
    )

    # q,k columns for this group, RoPE pair-permuted (2i,2i+1) -> (i,64+i)
    perm = np.empty(P, dtype=np.int64)
    perm[:64] = np.arange(0, P, 2)
    perm[64:] = np.arange(1, P, 2)
    wq = w_attn[:, gc : gc + G * P].reshape(C, G, P)[:, :, perm]
    wk = w_attn[:, C + gc : C + gc + G * P].reshape(C, G, P)[:, :, perm]
    wqk_cols = np.concatenate(
        [wq.reshape(C, G * P), wk.reshape(C, G * P)], axis=1
    )  # (C, 2*G*128)
    wqk = np.ascontiguousarray(
        wqk_cols.reshape(KO, P, 2 * G, P).transpose(2, 1, 0, 3)
    )

    wv_cols = w_attn[:, 2 * C + gc : 2 * C + gc + G * P]  # (C, G*128)
    wv = np.ascontiguousarray(
        wv_cols.reshape(KO, P, NV, VN).transpose(2, 1, 0, 3)
    )

    wp_rows = w_proj[gc : gc + G * P, :]  # (G*128, C)
    wp = np.ascontiguousarray(
        wp_rows.reshape(G, P, KO, P).transpose(2, 1, 0, 3)
    )

    cT = rope_cos[:T].T  # (64, T)
    sT = rope_sin[:T].T
    cospT = np.ascontiguousarray(np.concatenate([cT, cT], axis=0))  # (128, T)
    sinpT = np.ascontiguousarray(np.concatenate([-sT, sT], axis=0))
    mask = np.triu(np.ones((P, P), dtype=np.float32))

    return {
        "xt": xt.astype(np.float32),
        "wqk": wqk.astype(np.float32),
        "wv": wv.astype(np.float32),
        "wp": wp.astype(np.float32),
        "cosp": cospT.astype(np.float32),
        "sinp": sinpT.astype(np.float32),
        "maskt": mask,
        "ones_s": np.ones((P, 1), dtype=np.float32),
        "ones1": np.ones((1, P), dtype=np.float32),
    }


_NC_CACHE = {}
TRACE = False
LAST_RESULTS = None


def kernel(x, w_attn, w_proj, rope_cos, rope_sin):
    from concourse.bass_utils import run_bass_kernel_spmd

    x = np.asarray(x, dtype=np.float32)
    w_attn = np.asarray(w_attn, dtype=np.float32)
    w_proj = np.asarray(w_proj, dtype=np.float32)
    rope_cos = np.asarray(rope_cos, dtype=np.float32)
    rope_sin = np.asarray(rope_sin, dtype=np.float32)

    B, T, C = x.shape
    G = 8  # heads per group (16 heads / 2 groups)

    key = (T, C, G)
    if key not in _NC_CACHE:
        _NC_CACHE[key] = build_attention_core(T=T, C=C, G=G, n_half=2)
    nc = _NC_CACHE[key]

    in_maps = []
    for core in range(8):
        b, g = core // 2, core % 2
        in_maps.append(
            _prep_core_inputs(x[b], w_attn, w_proj, rope_cos, rope_sin, g, G=G)
        )

    res = run_bass_kernel_spmd(nc, in_maps, list(range(8)), trace=TRACE)
    global LAST_RESULTS
    LAST_RESULTS = res

    y = np.empty((B, T, C), dtype=np.float32)
    for b in range(B):
        acc = res.results[2 * b]["outT"] + res.results[2 * b + 1]["outT"]
        y[b] = acc.T
    return y



# revision 3
# speedup vs baseline: 1.7808x; 1.7808x over previous
"""Causal self-attention with RoPE on 8 Trainium2 NeuronCores.

Problem (hardcoded): x (4, 2048, 2048) f32, w_attn (2048, 6144),
w_proj (2048, 2048), rope_cos/rope_sin (2048, 64), 16 heads, hd=128.

Sharding: 8 cores = 4 batches x 2 head-groups (8 heads each).  Each core
computes qkv projection for its heads, RoPE, causal attention, and a
partial output projection (its head-group's rows of w_proj).  The host
sums the two partials per batch (the "all-reduce after c_proj") and
transposes back, since the device kernel works fully transposed.

Device layout choices:
  - qT, kT stored [hd=128 partitions, T free]; S^T tiles [j_keys, q]
    come straight from matmul(lhsT=kT_j, rhs=qT_q).  Softmax exp is
    elementwise (no max subtraction needed: scores ~ N(0,1), max ~ 6);
    causality = skipping j>q blocks + masking diagonal blocks.  The PV
    matmul consumes P^T directly with v in natural [T, hd] layout as
    lhsT, producing o^T with no transposes anywhere.
  - Softmax denominators accumulate on the PE alongside PV: a [1, QW]
    PSUM tile gets ones^T @ P^T per j-tile.  The reciprocal runs on one
    lane ([1, QW] on DVE), is broadcast to 128 lanes by a rank-1 matmul,
    and the final scale of o^T is deferred by one Q-chunk so the PE
    never waits on the ACT/DVE normalization chain.
  - RoPE pairs (2i, 2i+1) are host-permuted to (i, 64+i) by permuting
    w_attn's q/k columns (dot products are permutation invariant), so
    the rotation acts on contiguous partition ranges.
  - All matmuls run in float32r (fp32 bits, full-rate PE mode,
    ~1.5e-4 scale-relative error measured on K=2048).
"""

import sys

sys.path.insert(0, "/opt/trn_rl_repo")

import numpy as np

import concourse.bass as bass
import concourse.mybir as mybir
import concourse.tile as tile

F32 = mybir.dt.float32
F32R = mybir.dt.float32r
P = 128


# --------------------------------------------------------------------------
# This container's walrus build rejects any instruction carrying more than
# one sem wait.  Split extras onto NoOps inserted before the instruction on
# the same engine (per-engine program order makes the waits complete first).
def _split_multi_waits(nc):
    n = 0
    for fn in nc.m.functions:
        for bb in fn.blocks:
            out = []
            changed = False
            for inst in bb.instructions:
                si = inst.sync_info
                waits = list(si.on_wait or []) if si is not None else []
                if len(waits) > 1:
                    changed = True
                    n += 1
                    for w in waits[:-1]:
                        nop = mybir.InstNoOp(
                            name=nc.get_next_instruction_name(),
                            engine=inst.engine,
                            ins=[],
                            outs=[],
                            sync_info=mybir.SyncInfo(on_wait=[w], on_update=[]),
                        )
                        try:
                            nc.register_instruction(nop, overwrite=True)
                        except Exception:
                            pass
                        out.append(nop)
                    inst.sync_info = mybir.SyncInfo(
                        on_wait=[waits[-1]], on_update=list(si.on_update or [])
                    )
                out.append(inst)
            if changed:
                bb.instructions = out
    return n


def build_attention_core(T=2048, C=2048, G=8, n_half=2):
    """One core's program.  T tokens, C model dim, G heads in this core's
    group (hd=128 each).  Returns the Bass object."""
    KO = C // P          # contraction tiles over model dim
    TH = T // n_half     # tokens per phase-A pass
    NTC = max(TH // 512, 1)   # 512-wide t chunks per half (phase A qk)
    TCW = min(TH, 512)
    NTB = TH // P        # 128-tall t blocks per half (phase A v)
    VN = min(512, G * P)  # v column chunk
    NV = (G * P) // VN
    NQ = max(T // 512, 1)  # 512-wide q chunks (phase B)
    QW = min(T, 512)
    JPQ = QW // P        # j tiles per q chunk width
    NJ = T // P          # total j tiles
    KQ = max(KO // 4, 1)  # kc per xt quarter tile
    NXQ = KO // KQ

    nc = bass.Bass()
    xt = nc.dram_tensor("xt", [n_half, P, KO, TH], F32R, kind="ExternalInput")
    wqk = nc.dram_tensor("wqk", [2 * G, P, KO, P], F32R, kind="ExternalInput")
    wv = nc.dram_tensor("wv", [NV, P, KO, VN], F32R, kind="ExternalInput")
    wp = nc.dram_tensor("wp", [KO, P, G, P], F32R, kind="ExternalInput")
    # cosd = [cos; cos], sind = [-sin; +sin]  (rope = raw*cosd + swapped*sind)
    cosp = nc.dram_tensor("cosp", [P, T], F32, kind="ExternalInput")
    sinp = nc.dram_tensor("sinp", [P, T], F32, kind="ExternalInput")
    maskt = nc.dram_tensor("maskt", [P, P], F32, kind="ExternalInput")
    ones_s = nc.dram_tensor("ones_s", [P, 1], F32R, kind="ExternalInput")
    ones1 = nc.dram_tensor("ones1", [1, P], F32R, kind="ExternalInput")
    outT = nc.dram_tensor("outT", [C, T], F32, kind="ExternalOutput")

    scale = 1.0 / np.sqrt(128.0)

    with tile.TileContext(nc) as tc:
        with (
            tc.tile_pool(name="dram", bufs=1, space="DRAM") as dram,
            tc.tile_pool(name="const", bufs=1) as cpool,
        ):
            qkd = dram.tile([2 * G, P, T], F32R)
            od = dram.tile([G, P, T], F32R)

            with tc.tile_pool(name="vall", bufs=1) as va_pool:
                # v stays resident in SBUF through phases A and B:
                # v_all[ti, to, hh*128+d] = v[to*128+ti, head hh, d]
                v_all = va_pool.tile([P, NJ, G * P], F32R, tag="vall")

                # ------------- Phase A: qkT + RoPE, v -------------
                with (
                    tc.tile_pool(name="xt", bufs=4) as xt_pool,
                    tc.tile_pool(name="wqk", bufs=2) as wqk_pool,
                    tc.tile_pool(name="wv", bufs=1) as wv_pool,
                    tc.tile_pool(name="qkraw", bufs=1) as qkraw_pool,
                    tc.tile_pool(name="roped", bufs=1) as roped_pool,
                    tc.tile_pool(name="ropetmp", bufs=1) as rtmp_pool,
                    tc.tile_pool(name="psA", bufs=2, space="PSUM") as psA,
                    tc.tile_pool(name="psV", bufs=2, space="PSUM") as psV,
                ):
                    # Load the first half's x and first weights before the
                    # (large, non-urgent) constants so the PE starts ASAP.
                    xtq_next = []
                    for qq in range(NXQ):
                        xq = xt_pool.tile([P, KQ, TH], F32R, tag="xtq",
                                          name=f"xtq{qq}")
                        eng = nc.sync if qq % 2 == 0 else nc.scalar
                        eng.dma_start(xq[:], xt[0, :, qq * KQ : (qq + 1) * KQ, :])
                        xtq_next.append(xq)

                    cos_s = cpool.tile([P, T], F32)
                    nc.gpsimd.dma_start(cos_s[:], cosp[:])
                    sin_s = cpool.tile([P, T], F32)
                    nc.gpsimd.dma_start(sin_s[:], sinp[:])
                    mask_s = cpool.tile([P, P], F32)
                    nc.gpsimd.dma_start(mask_s[:], maskt[:])
                    one_col = cpool.tile([P, 1], F32R)
                    nc.gpsimd.dma_start(one_col[:], ones_s[:])
                    one_row = cpool.tile([1, P], F32R)
                    nc.gpsimd.dma_start(one_row[:], ones1[:])

                    for H in range(n_half):
                        t0 = H * TH
                        xtq = xtq_next
                        if H + 1 < n_half:
                            xtq_next = []
                            for qq in range(NXQ):
                                xq = xt_pool.tile([P, KQ, TH], F32R, tag="xtq",
                                                  name=f"xtq{qq}")
                                eng = nc.sync if qq % 2 == 0 else nc.scalar
                                eng.dma_start(
                                    xq[:],
                                    xt[H + 1, :, qq * KQ : (qq + 1) * KQ, :],
                                )
                                xtq_next.append(xq)
                        # --- q,k heads ---
                        for m in range(2 * G):
                            w_s = wqk_pool.tile([P, KO, P], F32R, tag="wqk")
                            nc.scalar.dma_start(w_s[:], wqk[m])
                            pss = [
                                psA.tile([P, TCW], F32, tag=f"pqk{i}",
                                         name=f"pqk{i}")
                                for i in range(NTC)
                            ]
                            for kc in range(KO):
                                for i in range(NTC):
                                    nc.tensor.matmul(
                                        pss[i][:],
                                        w_s[:, kc, :],
                                        xtq[kc // KQ][
                                            :, kc % KQ,
                                            i * TCW : (i + 1) * TCW,
                                        ],
                                        start=(kc == 0),
                                        stop=(kc == KO - 1),
                                    )
                            # RoPE: rope = raw*[cos;cos] + swap(raw)*[-sin;+sin]
                            raw = qkraw_pool.tile([P, TH], F32, tag="qkraw")
                            sw = rtmp_pool.tile([P, TH], F32, tag="rtmp")
                            rop = roped_pool.tile([P, TH], F32R, tag="roped")
                            for i in range(NTC):
                                sl = slice(i * TCW, (i + 1) * TCW)
                                nc.any.tensor_copy(raw[:, sl], pss[i][:])
                                nc.vector.tensor_mul(
                                    rop[:, sl], pss[i][:],
                                    cos_s[:, t0 + i * TCW : t0 + (i + 1) * TCW],
                                )
                            nc.gpsimd.dma_start(sw[0:64, :], raw[64:128, :])
                            nc.gpsimd.dma_start(sw[64:128, :], raw[0:64, :])
                            nc.vector.tensor_mul(
                                sw[:], sw[:], sin_s[:, t0 : t0 + TH]
                            )
                            nc.vector.tensor_add(rop[:], rop[:], sw[:])
                            nc.gpsimd.dma_start(qkd[m, :, t0 : t0 + TH], rop[:])
                        # --- v (kept in SBUF, natural [t, d] layout) ---
                        for n2 in range(NV):
                            wv_s = wv_pool.tile([P, KO, VN], F32R, tag="wv")
                            nc.scalar.dma_start(wv_s[:], wv[n2])
                            for tb in range(NTB):
                                psv = psV.tile([P, VN], F32, tag="pv")
                                for kc in range(KO):
                                    nc.tensor.matmul(
                                        psv[:],
                                        xtq[kc // KQ][
                                            :, kc % KQ, tb * P : (tb + 1) * P
                                        ],
                                        wv_s[:, kc, :],
                                        start=(kc == 0),
                                        stop=(kc == KO - 1),
                                    )
                                nc.any.tensor_copy(
                                    v_all[
                                        :, H * NTB + tb,
                                        n2 * VN : (n2 + 1) * VN,
                                    ],
                                    psv[:],
                                )

                # ------------- Phase B: attention per head -------------
                with (
                    tc.tile_pool(name="qh", bufs=3) as q_pool,
                    tc.tile_pool(name="kh", bufs=3) as k_pool,
                    tc.tile_pool(name="pt", bufs=6) as pt_pool,
                    tc.tile_pool(name="rsb", bufs=4) as rsb_pool,
                    tc.tile_pool(name="rinv", bufs=2) as rinv_pool,
                    tc.tile_pool(name="oacc", bufs=2) as oacc_pool,
                    tc.tile_pool(name="psS", bufs=3, space="PSUM") as psS,
                    tc.tile_pool(name="psO", bufs=2, space="PSUM") as psO,
                    tc.tile_pool(name="psR", bufs=2, space="PSUM") as psR,
                    tc.tile_pool(name="psRep", bufs=1, space="PSUM") as psRep,
                ):
                    for h in range(G):
                        qT = q_pool.tile([P, T], F32R, tag="q")
                        nc.sync.dma_start(qT[:], qkd[h])
                        kT = k_pool.tile([P, T], F32R, tag="k")
                        nc.sync.dma_start(kT[:], qkd[G + h])
                        oT = oacc_pool.tile([P, T], F32R, tag="oacc")

                        # deferred normalization state: (Q, rqr tile)
                        norm_pend = []

                        def emit_norm(oT=oT):
                            Qp, rqr_p = norm_pend.pop(0)
                            psrep = psRep.tile([P, QW], F32, tag="psrep")
                            nc.tensor.matmul(
                                psrep[:], one_row[:], rqr_p[:],
                                start=True, stop=True, skip_group_check=True,
                            )
                            rinv = rinv_pool.tile([P, QW], F32, tag="rinv")
                            nc.any.tensor_copy(rinv[:], psrep[:])
                            nc.vector.tensor_mul(
                                oT[:, Qp * QW : (Qp + 1) * QW],
                                oT[:, Qp * QW : (Qp + 1) * QW],
                                rinv[:],
                            )

                        for Q in range(NQ):
                            jmax = JPQ * (Q + 1) - 1  # inclusive
                            pso = psO.tile([P, QW], F32, tag="pso")
                            psr = psR.tile([1, QW], F32, tag="psr")
                            # PV(J) consumes exp(S(J)) from ACT; emit it LOOK
                            # S-matmuls later so PE never stalls on ACT.
                            LOOK = 2
                            pend = []
                            for J in range(jmax + 1 + LOOK):
                                if J <= jmax:
                                    k_d = J - JPQ * Q  # diag idx if >= 0
                                    co = max(k_d, 0) * P
                                    pss = psS.tile([P, QW], F32, tag="pss")
                                    nc.tensor.matmul(
                                        pss[:, co:],
                                        kT[:, J * P : (J + 1) * P],
                                        qT[:, Q * QW + co : (Q + 1) * QW],
                                        start=True,
                                        stop=True,
                                        skip_group_check=True,
                                    )
                                    pT = pt_pool.tile([P, QW], F32R, tag="pt")
                                    nc.scalar.activation(
                                        pT[:, co:], pss[:, co:],
                                        mybir.ActivationFunctionType.Exp,
                                        scale=scale,
                                    )
                                    if k_d >= 0:
                                        nc.vector.tensor_mul(
                                            pT[:, co : co + P],
                                            pT[:, co : co + P],
                                            mask_s[:],
                                        )
                                    pend.append((J, co, pT))
                                if J >= LOOK:
                                    Jp, cop, pTp = pend.pop(0)
                                    nc.tensor.matmul(
                                        pso[:, cop:],
                                        v_all[:, Jp, h * P : (h + 1) * P],
                                        pTp[:, cop:],
                                        start=(Jp == 0),
                                        stop=(Jp == jmax),
                                        skip_group_check=True,
                                    )
                                    nc.tensor.matmul(
                                        psr[:, cop:],
                                        one_col[:],
                                        pTp[:, cop:],
                                        start=(Jp == 0),
                                        stop=(Jp == jmax),
                                        skip_group_check=True,
                                    )
                            # copy raw o^T out of PSUM; reciprocal of the
                            # accumulated row sums (one lane) — both off the
                            # PE critical path.
                            nc.any.tensor_copy(
                                oT[:, Q * QW : (Q + 1) * QW], pso[:]
                            )
                            rsb = rsb_pool.tile([1, QW], F32, tag="rsb")
                            nc.any.tensor_copy(rsb[:], psr[:])
                            rqr = rsb_pool.tile([1, QW], F32R, tag="rqr",
                                                name="rqr")
                            with nc.allow_low_precision(reason="f32r is 4B"):
                                nc.vector.reciprocal(rqr[:], rsb[:])
                            norm_pend.append((Q, rqr))
                            # normalize the PREVIOUS Q-chunk now: its
                            # broadcast matmul lands between this chunk's
                            # matmuls, after its chain has already resolved.
                            if Q > 0:
                                emit_norm()
                        emit_norm()
                        nc.scalar.dma_start(od[h], oT[:])

            # ------------- Phase C: output projection -------------
            with (
                tc.tile_pool(name="wp", bufs=1) as wp_pool,
                tc.tile_pool(name="otc", bufs=2) as otc_pool,
                tc.tile_pool(name="csb", bufs=4) as csb_pool,
                tc.tile_pool(name="psC", bufs=4, space="PSUM") as psC,
            ):
                wp_s = wp_pool.tile([P, KO, G, P], F32R, tag="wp")
                for m in range(KO):
                    nc.sync.dma_start(
                        wp_s[:, m], wp[m]
                    )
                for t in range(NQ):
                    oTt = otc_pool.tile([P, G, QW], F32R, tag="otc")
                    nc.sync.dma_start(
                        oTt[:],
                        od.rearrange("h p t -> p h t")[
                            :, :, t * QW : (t + 1) * QW
                        ],
                    )
                    for m in range(KO):
                        psc = psC.tile([P, QW], F32, tag="psc")
                        for hh in range(G):
                            nc.tensor.matmul(
                                psc[:],
                                wp_s[:, m, hh, :],
                                oTt[:, hh, :],
                                start=(hh == 0),
                                stop=(hh == G - 1),
                            )
                        csb = csb_pool.tile([P, QW], F32, tag="csb")
                        nc.any.tensor_copy(csb[:], psc[:])
                        nc.sync.dma_start(
                            outT[m * P : (m + 1) * P, t * QW : (t + 1) * QW],
                            csb[:],
                        )

    _split_multi_waits(nc)
    return nc


# --------------------------------------------------------------------------
def _prep_core_inputs(xb, w_attn, w_proj, rope_cos, rope_sin, g, G=8, n_half=2):
    """Host-side shard prep for one core: batch slice xb (T, C), group g."""
    T, C = xb.shape
    KO = C // P
    TH = T // n_half
    VN = min(512, G * P)
    NV = (G * P) // VN
    gc = g * G * P  # column offset of this group within one qkv section

    # x^T arranged [half, ki, ko, t]
    xtT = np.ascontiguousarray(xb.T)  # (C, T)
    xt = np.ascontiguousarray(
        xtT.reshape(KO, P, n_half, TH).transpose(2, 1, 0, 3)
    )

    # q,k columns for this group, RoPE pair-permuted (2i,2i+1) -> (i,64+i)
    perm = np.empty(P, dtype=np.int64)
    perm[:64] = np.arange(0, P, 2)
    perm[64:] = np.arange(1, P, 2)
    wq = w_attn[:, gc : gc + G * P].reshape(C, G, P)[:, :, perm]
    wk = w_attn[:, C + gc : C + gc + G * P].reshape(C, G, P)[:, :, perm]
    wqk_cols = np.concatenate(
        [wq.reshape(C, G * P), wk.reshape(C, G * P)], axis=1
    )  # (C, 2*G*128)
    wqk = np.ascontiguousarray(
        wqk_cols.reshape(KO, P, 2 * G, P).transpose(2, 1, 0, 3)
    )

    wv_cols = w_attn[:, 2 * C + gc : 2 * C + gc + G * P]  # (C, G*128)
    wv = np.ascontiguousarray(
        wv_cols.reshape(KO, P, NV, VN).transpose(2, 1, 0, 3)
    )

    wp_rows = w_proj[gc : gc + G * P, :]  # (G*128, C)
    wp = np.ascontiguousarray(
        wp_rows.reshape(G, P, KO, P).transpose(2, 1, 0, 3)
    )

    cT = rope_cos[:T].T  # (64, T)
    sT = rope_sin[:T].T
    cospT = np.ascontiguousarray(np.concatenate([cT, cT], axis=0))  # (128, T)
    sinpT = np.ascontiguousarray(np.concatenate([-sT, sT], axis=0))
    mask = np.triu(np.ones((P, P), dtype=np.float32))

    return {
        "xt": xt.astype(np.float32),
        "wqk": wqk.astype(np.float32),
        "wv": wv.astype(np.float32),
        "wp": wp.astype(np.float32),
        "cosp": cospT.astype(np.float32),
        "sinp": sinpT.astype(np.float32),
        "maskt": mask,
        "ones_s": np.ones((P, 1), dtype=np.float32),
        "ones1": np.ones((1, P), dtype=np.float32),
    }


_NC_CACHE = {}
TRACE = False
LAST_RESULTS = None


def kernel(x, w_attn, w_proj, rope_cos, rope_sin):
    from concourse.bass_utils import run_bass_kernel_spmd

    x = np.asarray(x, dtype=np.float32)
    w_attn = np.asarray(w_attn, dtype=np.float32)
    w_proj = np.asarray(w_proj, dtype=np.float32)
    rope_cos = np.asarray(rope_cos, dtype=np.float32)
    rope_sin = np.asarray(rope_sin, dtype=np.float32)

    B, T, C = x.shape
    G = 8  # heads per group (16 heads / 2 groups)

    key = (T, C, G)
    if key not in _NC_CACHE:
        _NC_CACHE[key] = build_attention_core(T=T, C=C, G=G, n_half=2)
    nc = _NC_CACHE[key]

    in_maps = []
    for core in range(8):
        b, g = core // 2, core % 2
        in_maps.append(
            _prep_core_inputs(x[b], w_attn, w_proj, rope_cos, rope_sin, g, G=G)
        )

    res = run_bass_kernel_spmd(nc, in_maps, list(range(8)), trace=TRACE)
    global LAST_RESULTS
    LAST_RESULTS = res

    y = np.empty((B, T, C), dtype=np.float32)
    for b in range(B):
        acc = res.results[2 * b]["outT"] + res.results[2 * b + 1]["outT"]
        y[b] = acc.T
    return y
